# revision 19
# baseline (speedup 1.0000x reference)
"""PatchCore anomaly score kernel for 8 trn2 NeuronCores.

score = sqrt(max_n min_m ||patches[n] - memory_bank[m]||^2)

Device (per core, memory_bank sharded 4096 rows/core):
  r_c[n] = max_m (2*patches[n].bank[m] - (m_sq[m] - C))     [MAX-only ops]
Host:
  min_d2[n] = p_sq[n] + C - max_c r_c[n];  score = sqrt(max_n min_d2)
"""

import sys

import numpy as np

try:
    import concourse.bass as bass
except ImportError:
    sys.path.insert(0, "/opt/trn_rl_repo")
    import concourse.bass as bass

import concourse.bacc as bacc
import concourse.tile as tile
from concourse import mybir
from concourse.bass_utils import run_bass_kernel_spmd

import ml_dtypes

N = 8192          # patches
M_TOTAL = 32768   # memory bank rows
D = 512           # feature dim
N_CORES = 8
M = M_TOTAL // N_CORES   # 4096 bank rows per core

KP = 4            # k-chunks of 128 over D
NT = N // 512     # 16 n-tiles of 512 patches
MT = M // 128     # 32 m-tiles of 128 bank rows


def _build_nc(repeat=1, fp8=True):
    # Bacc (not Bass): its compile() pass splits multi-sem waits into
    # event semaphores — TRN2 allows only 1 embedded wait per instruction.
    # repeat>1 wraps the compute in a For_i hardware loop (bench-only:
    # amplifies device work so wall-clock deltas resolve the kernel time).
    if fp8:
        return _build_nc_fp8(repeat)
    nc = bacc.Bacc(None, target_bir_lowering=False)
    f32 = mybir.dt.float32
    bf16 = mybir.dt.bfloat16

    at_d = nc.dram_tensor("at", [D, N], bf16, kind="ExternalInput")
    bt_d = nc.dram_tensor("bt", [D, M], bf16, kind="ExternalInput")
    msq_d = nc.dram_tensor("msq", [128, MT], f32, kind="ExternalInput")
    id_d = nc.dram_tensor("ident", [128, 128], bf16, kind="ExternalInput")
    # out[p, blk] = r[blk*128 + p]; host transposes
    out_d = nc.dram_tensor("out", [128, NT * 4], f32, kind="ExternalOutput")

    with tile.TileContext(nc) as tc:
        with (
            tc.tile_pool(name="at", bufs=1) as at_pool,
            tc.tile_pool(name="bt", bufs=1) as bt_pool,
            tc.tile_pool(name="msq", bufs=1) as msq_pool,
            tc.tile_pool(name="rmax", bufs=2) as rmax_pool,
            tc.tile_pool(name="evac", bufs=4) as evac_pool,
            tc.tile_pool(name="res", bufs=1) as res_pool,
            tc.tile_pool(name="psum", bufs=6, space="PSUM") as psum_pool,
            tc.tile_pool(name="psumt", bufs=2, space="PSUM") as psumt_pool,
        ):
            msq_t = msq_pool.tile([128, MT], f32)
            nc.gpsimd.dma_start(msq_t[:], msq_d[:])
            id_t = msq_pool.tile([128, 128], bf16, name="id_t")
            nc.gpsimd.dma_start(id_t[:], id_d[:])
            res_t = res_pool.tile([128, NT * 4], f32)

            # bank first (whole bank needed for n-tile 0), in k/col chunks
            bt_t = [bt_pool.tile([128, M], bf16, name=f"bt{k}") for k in range(KP)]
            for k in range(KP):
                for j in range(4):
                    nc.gpsimd.dma_start(
                        bt_t[k][:, bass.ts(j, M // 4)],
                        bt_d[bass.ts(k, 128), bass.ts(j, M // 4)],
                    )
            # patches, in column chunks matching n-tile consumption order
            at_t = [at_pool.tile([128, N], bf16, name=f"at{k}") for k in range(KP)]
            for j in range(8):
                for k in range(KP):
                    nc.gpsimd.dma_start(
                        at_t[k][:, bass.ts(j, N // 8)],
                        at_d[bass.ts(k, 128), bass.ts(j, N // 8)],
                    )

            def reduce_ntile(n, rmax):
                # partition-axis max of rmax [128,512] via PE transpose
                # of each 128-col chunk + DVE free-axis max reduce.
                for c in range(4):
                    pst = psumt_pool.tile([128, 128], bf16, name="pst")
                    nc.tensor.transpose(
                        pst[:], rmax[:, bass.ts(c, 128)], id_t[:]
                    )
                    col = n * 4 + c
                    nc.vector.tensor_reduce(
                        res_t[:, col : col + 1], pst[:],
                        mybir.AxisListType.X, mybir.AluOpType.max,
                    )

            def compute_body():
                prev = None
                for n in range(NT):
                    rmax = rmax_pool.tile([128, 512], bf16)
                    for m in range(MT):
                        ps = psum_pool.tile([128, 512], f32)
                        for k in range(KP):
                            nc.tensor.matmul(
                                ps[:],
                                bt_t[k][:, bass.ts(m, 128)],
                                at_t[k][:, bass.ts(n, 512)],
                                start=(k == 0),
                                stop=(k == KP - 1),
                            )
                        if m == 0:
                            nc.scalar.activation(
                                rmax[:], ps[:],
                                mybir.ActivationFunctionType.Identity,
                                bias=msq_t[:, m : m + 1], scale=2.0,
                            )
                        else:
                            ev = evac_pool.tile([128, 512], bf16)
                            nc.scalar.activation(
                                ev[:], ps[:],
                                mybir.ActivationFunctionType.Identity,
                                bias=msq_t[:, m : m + 1], scale=2.0,
                            )
                            nc.vector.tensor_max(rmax[:], rmax[:], ev[:])
                        if m == 4 and prev is not None:
                            reduce_ntile(n - 1, prev)  # keep PE busy before stall
                    prev = rmax
                reduce_ntile(NT - 1, prev)

            if repeat == 1:
                compute_body()
            else:
                with tc.For_i(0, repeat):
                    compute_body()
            nc.gpsimd.dma_start(out_d[:], res_t[:])

    nc.finalize()
    return nc


def _build_nc_fp8(repeat=1):
    """fp8e4m3 DoubleRow variant: K=256 per matmul, 2x PE throughput.

    Device result is approximate; kernel() refines the top candidates
    exactly on host (inputs are small enough that a few exact rows of
    d2 cost ~0.1s in BLAS).
    """
    nc = bacc.Bacc(None, target_bir_lowering=False)
    f32 = mybir.dt.float32
    bf16 = mybir.dt.bfloat16
    fp8 = mybir.dt.float8e4

    # dim1 index ci = d//128; slice [:, 2c:2c+2, :] = K rows [c*256,(c+1)*256)
    at_d = nc.dram_tensor("at", [128, KP, N], fp8, kind="ExternalInput")
    bt_d = nc.dram_tensor("bt", [128, KP, M], fp8, kind="ExternalInput")
    msq_d = nc.dram_tensor("msq", [128, MT], f32, kind="ExternalInput")
    id_d = nc.dram_tensor("ident", [128, 128], bf16, kind="ExternalInput")
    out_d = nc.dram_tensor("out", [128, NT * 4], f32, kind="ExternalOutput")

    with tile.TileContext(nc) as tc:
        with (
            tc.tile_pool(name="at", bufs=1) as at_pool,
            tc.tile_pool(name="bt", bufs=1) as bt_pool,
            tc.tile_pool(name="msq", bufs=1) as msq_pool,
            tc.tile_pool(name="rmax", bufs=2) as rmax_pool,
            tc.tile_pool(name="evac", bufs=4) as evac_pool,
            tc.tile_pool(name="res", bufs=1) as res_pool,
            tc.tile_pool(name="psum", bufs=6, space="PSUM") as psum_pool,
            tc.tile_pool(name="psumt", bufs=2, space="PSUM") as psumt_pool,
        ):
            msq_t = msq_pool.tile([128, MT], f32)
            nc.gpsimd.dma_start(msq_t[:], msq_d[:])
            id_t = msq_pool.tile([128, 128], bf16, name="id_t")
            nc.gpsimd.dma_start(id_t[:], id_d[:])
            res_t = res_pool.tile([128, NT * 4], f32)

            bt_t = bt_pool.tile([128, KP, M], fp8)
            for ci in range(KP):
                for j in range(4):
                    nc.gpsimd.dma_start(
                        bt_t[:, ci, bass.ts(j, M // 4)],
                        bt_d[:, ci, bass.ts(j, M // 4)],
                    )
            at_t = at_pool.tile([128, KP, N], fp8)
            for j in range(8):
                for ci in range(KP):
                    nc.gpsimd.dma_start(
                        at_t[:, ci, bass.ts(j, N // 8)],
                        at_d[:, ci, bass.ts(j, N // 8)],
                    )

            def reduce_ntile(n, rmax):
                for c in range(4):
                    pst = psumt_pool.tile([128, 128], bf16, name="pst")
                    nc.tensor.transpose(
                        pst[:], rmax[:, bass.ts(c, 128)], id_t[:]
                    )
                    col = n * 4 + c
                    nc.vector.tensor_reduce(
                        res_t[:, col : col + 1], pst[:],
                        mybir.AxisListType.X, mybir.AluOpType.max,
                    )

            def compute_body():
                prev = None
                for n in range(NT):
                    rmax = rmax_pool.tile([128, 512], bf16)
                    for m in range(MT):
                        ps = psum_pool.tile([128, 512], f32)
                        for c in range(2):
                            nc.tensor.matmul(
                                ps[:],
                                bt_t[:, 2 * c : 2 * c + 2, bass.ts(m, 128)],
                                at_t[:, 2 * c : 2 * c + 2, bass.ts(n, 512)],
                                start=(c == 0),
                                stop=(c == 1),
                                perf_mode=mybir.MatmulPerfMode.DoubleRow,
                            )
                        if m == 0:
                            nc.scalar.activation(
                                rmax[:], ps[:],
                                mybir.ActivationFunctionType.Identity,
                                bias=msq_t[:, m : m + 1], scale=2.0,
                            )
                        else:
                            ev = evac_pool.tile([128, 512], bf16)
                            nc.scalar.activation(
                                ev[:], ps[:],
                                mybir.ActivationFunctionType.Identity,
                                bias=msq_t[:, m : m + 1], scale=2.0,
                            )
                            nc.vector.tensor_max(rmax[:], rmax[:], ev[:])
                        if m == 4 and prev is not None:
                            reduce_ntile(n - 1, prev)
                    prev = rmax
                reduce_ntile(NT - 1, prev)

            if repeat == 1:
                compute_body()
            else:
                with tc.For_i(0, repeat):
                    compute_body()
            nc.gpsimd.dma_start(out_d[:], res_t[:])

    nc.finalize()
    return nc


_NC = None


def prepare_in_maps(patches: np.ndarray, memory_bank: np.ndarray, fp8=True):
    m_sq = np.sum(
        memory_bank.astype(np.float64) ** 2, axis=1
    )
    C = float(np.mean(m_sq))
    id_np = np.eye(128, dtype=ml_dtypes.bfloat16)
    if fp8:
        # [128, 4, N] with dim1 = d//128 (K-chunk index)
        at_np = np.ascontiguousarray(
            patches.T.astype(ml_dtypes.float8_e4m3)
            .reshape(KP, 128, N).transpose(1, 0, 2)
        )
    else:
        at_np = np.ascontiguousarray(patches.T).astype(ml_dtypes.bfloat16)
    in_maps = []
    for c in range(N_CORES):
        bank_c = memory_bank[c * M : (c + 1) * M]
        if fp8:
            bt_np = np.ascontiguousarray(
                bank_c.T.astype(ml_dtypes.float8_e4m3)
                .reshape(KP, 128, M).transpose(1, 0, 2)
            )
        else:
            bt_np = np.ascontiguousarray(bank_c.T).astype(ml_dtypes.bfloat16)
        msq_c = -(m_sq[c * M : (c + 1) * M] - C)
        msq_np = np.ascontiguousarray(
            msq_c.reshape(MT, 128).T
        ).astype(np.float32)
        in_maps.append({"at": at_np, "bt": bt_np, "msq": msq_np, "ident": id_np})
    return in_maps


def kernel(patches: np.ndarray, memory_bank: np.ndarray) -> np.ndarray:
    global _NC
    if _NC is None:
        _NC = _build_nc()
    nc = _NC

    p64 = patches.astype(np.float64)
    b64 = memory_bank.astype(np.float64)
    p_sq = np.sum(p64 * p64, axis=1)          # [N]
    m_sq = np.sum(b64 * b64, axis=1)          # [M_TOTAL]
    C = float(np.mean(m_sq))

    in_maps = prepare_in_maps(patches, memory_bank)

    br = run_bass_kernel_spmd(nc, in_maps, list(range(N_CORES)))
    r = np.max(
        np.stack(
            [np.asarray(br.results[c]["out"], np.float64).T.reshape(N)
             for c in range(N_CORES)]
        ),
        axis=0,
    )
    min_d2 = np.maximum(p_sq + C - r, 0.0)

    # Host refinement: device min_d2 is approximate (fp8 matmul + bf16 max
    # accumulation). Recompute exact d2 rows for every candidate patch whose
    # approx score is within EPS of the max; EPS covers the empirical device
    # error (~+-8 for fp8) with wide margin.
    EPS = 60.0
    amax = float(min_d2.max())
    S = np.flatnonzero(min_d2 >= amax - EPS)
    if len(S) > 2048:
        S = np.argsort(min_d2)[-2048:]
    cross_S = p64[S] @ b64.T
    d2_S = p_sq[S, None] + m_sq[None, :] - 2.0 * cross_S
    score = np.sqrt(max(float(np.maximum(d2_S, 0.0).min(axis=1).max()), 0.0))
    return np.asarray(score, dtype=np.float32)


# revision 20
# speedup vs baseline: 63949.1257x; 63949.1257x over previous
"""PatchCore anomaly score kernel for 8 trn2 NeuronCores.

score = sqrt(max_n min_m ||patches[n] - memory_bank[m]||^2)

Device (per core, memory_bank sharded 4096 rows/core):
  r_c[n] = max_m (2*patches[n].bank[m] - (m_sq[m] - C))     [MAX-only ops]
Host:
  min_d2[n] = p_sq[n] + C - max_c r_c[n];  score = sqrt(max_n min_d2)
"""

import sys

import numpy as np

try:
    import concourse.bass as bass
except ImportError:
    sys.path.insert(0, "/opt/trn_rl_repo")
    import concourse.bass as bass

import concourse.bacc as bacc
import concourse.tile as tile
from concourse import mybir
from concourse.bass_utils import run_bass_kernel_spmd

import ml_dtypes

N = 8192          # patches
M_TOTAL = 32768   # memory bank rows
D = 512           # feature dim
N_CORES = 8
M = M_TOTAL // N_CORES   # 4096 bank rows per core

KP = 4            # k-chunks of 128 over D
NT = N // 512     # 16 n-tiles of 512 patches
MT = M // 128     # 32 m-tiles of 128 bank rows


def _build_nc(repeat=1, fp8=True):
    # Bacc (not Bass): its compile() pass splits multi-sem waits into
    # event semaphores — TRN2 allows only 1 embedded wait per instruction.
    # repeat>1 wraps the compute in a For_i hardware loop (bench-only:
    # amplifies device work so wall-clock deltas resolve the kernel time).
    if fp8:
        return _build_nc_fp8(repeat)
    nc = bacc.Bacc(None, target_bir_lowering=False)
    f32 = mybir.dt.float32
    bf16 = mybir.dt.bfloat16

    at_d = nc.dram_tensor("at", [D, N], bf16, kind="ExternalInput")
    bt_d = nc.dram_tensor("bt", [D, M], bf16, kind="ExternalInput")
    msq_d = nc.dram_tensor("msq", [128, MT], f32, kind="ExternalInput")
    id_d = nc.dram_tensor("ident", [128, 128], bf16, kind="ExternalInput")
    # out[p, blk] = r[blk*128 + p]; host transposes
    out_d = nc.dram_tensor("out", [128, NT * 4], f32, kind="ExternalOutput")

    with tile.TileContext(nc) as tc:
        with (
            tc.tile_pool(name="at", bufs=1) as at_pool,
            tc.tile_pool(name="bt", bufs=1) as bt_pool,
            tc.tile_pool(name="msq", bufs=1) as msq_pool,
            tc.tile_pool(name="rmax", bufs=2) as rmax_pool,
            tc.tile_pool(name="evac", bufs=4) as evac_pool,
            tc.tile_pool(name="res", bufs=1) as res_pool,
            tc.tile_pool(name="psum", bufs=6, space="PSUM") as psum_pool,
            tc.tile_pool(name="psumt", bufs=2, space="PSUM") as psumt_pool,
        ):
            msq_t = msq_pool.tile([128, MT], f32)
            nc.gpsimd.dma_start(msq_t[:], msq_d[:])
            id_t = msq_pool.tile([128, 128], bf16, name="id_t")
            nc.gpsimd.dma_start(id_t[:], id_d[:])
            res_t = res_pool.tile([128, NT * 4], f32)

            # bank first (whole bank needed for n-tile 0), in k/col chunks
            bt_t = [bt_pool.tile([128, M], bf16, name=f"bt{k}") for k in range(KP)]
            for k in range(KP):
                for j in range(4):
                    nc.gpsimd.dma_start(
                        bt_t[k][:, bass.ts(j, M // 4)],
                        bt_d[bass.ts(k, 128), bass.ts(j, M // 4)],
                    )
            # patches, in column chunks matching n-tile consumption order
            at_t = [at_pool.tile([128, N], bf16, name=f"at{k}") for k in range(KP)]
            for j in range(8):
                for k in range(KP):
                    nc.gpsimd.dma_start(
                        at_t[k][:, bass.ts(j, N // 8)],
                        at_d[bass.ts(k, 128), bass.ts(j, N // 8)],
                    )

            def reduce_ntile(n, rmax):
                # partition-axis max of rmax [128,512] via PE transpose
                # of each 128-col chunk + DVE free-axis max reduce.
                for c in range(4):
                    pst = psumt_pool.tile([128, 128], bf16, name="pst")
                    nc.tensor.transpose(
                        pst[:], rmax[:, bass.ts(c, 128)], id_t[:]
                    )
                    col = n * 4 + c
                    nc.vector.tensor_reduce(
                        res_t[:, col : col + 1], pst[:],
                        mybir.AxisListType.X, mybir.AluOpType.max,
                    )

            def compute_body():
                prev = None
                for n in range(NT):
                    rmax = rmax_pool.tile([128, 512], bf16)
                    for m in range(MT):
                        ps = psum_pool.tile([128, 512], f32)
                        for k in range(KP):
                            nc.tensor.matmul(
                                ps[:],
                                bt_t[k][:, bass.ts(m, 128)],
                                at_t[k][:, bass.ts(n, 512)],
                                start=(k == 0),
                                stop=(k == KP - 1),
                            )
                        if m == 0:
                            nc.scalar.activation(
                                rmax[:], ps[:],
                                mybir.ActivationFunctionType.Identity,
                                bias=msq_t[:, m : m + 1], scale=2.0,
                            )
                        else:
                            ev = evac_pool.tile([128, 512], bf16)
                            nc.scalar.activation(
                                ev[:], ps[:],
                                mybir.ActivationFunctionType.Identity,
                                bias=msq_t[:, m : m + 1], scale=2.0,
                            )
                            nc.vector.tensor_max(rmax[:], rmax[:], ev[:])
                        if m == 4 and prev is not None:
                            reduce_ntile(n - 1, prev)  # keep PE busy before stall
                    prev = rmax
                reduce_ntile(NT - 1, prev)

            if repeat == 1:
                compute_body()
            else:
                with tc.For_i(0, repeat):
                    compute_body()
            nc.gpsimd.dma_start(out_d[:], res_t[:])

    nc.finalize()
    return nc


def _build_nc_fp8(repeat=1):
    """fp8e4m3 DoubleRow variant: K=256 per matmul, 2x PE throughput.

    Device result is approximate; kernel() refines the top candidates
    exactly on host (inputs are small enough that a few exact rows of
    d2 cost ~0.1s in BLAS).
    """
    nc = bacc.Bacc(None, target_bir_lowering=False)
    f32 = mybir.dt.float32
    bf16 = mybir.dt.bfloat16
    fp8 = mybir.dt.float8e4

    # dim1 index ci = d//128; slice [:, 2c:2c+2, :] = K rows [c*256,(c+1)*256)
    at_d = nc.dram_tensor("at", [128, KP, N], fp8, kind="ExternalInput")
    bt_d = nc.dram_tensor("bt", [128, KP, M], fp8, kind="ExternalInput")
    msq_d = nc.dram_tensor("msq", [128, MT], f32, kind="ExternalInput")
    id_d = nc.dram_tensor("ident", [128, 128], bf16, kind="ExternalInput")
    out_d = nc.dram_tensor("out", [128, NT * 4], f32, kind="ExternalOutput")

    with tile.TileContext(nc) as tc:
        with (
            tc.tile_pool(name="at", bufs=1) as at_pool,
            tc.tile_pool(name="bt", bufs=1) as bt_pool,
            tc.tile_pool(name="msq", bufs=1) as msq_pool,
            tc.tile_pool(name="rmax", bufs=2) as rmax_pool,
            tc.tile_pool(name="evac", bufs=4) as evac_pool,
            tc.tile_pool(name="res", bufs=1) as res_pool,
            tc.tile_pool(name="psum", bufs=6, space="PSUM") as psum_pool,
            tc.tile_pool(name="psumt", bufs=2, space="PSUM") as psumt_pool,
        ):
            msq_t = msq_pool.tile([128, MT], f32)
            nc.gpsimd.dma_start(msq_t[:], msq_d[:])
            id_t = msq_pool.tile([128, 128], bf16, name="id_t")
            nc.gpsimd.dma_start(id_t[:], id_d[:])
            res_t = res_pool.tile([128, NT * 4], f32)

            bt_t = bt_pool.tile([128, KP, M], fp8)
            for ci in range(KP):
                for j in range(4):
                    nc.gpsimd.dma_start(
                        bt_t[:, ci, bass.ts(j, M // 4)],
                        bt_d[:, ci, bass.ts(j, M // 4)],
                    )
            at_t = at_pool.tile([128, KP, N], fp8)
            for j in range(8):
                for ci in range(KP):
                    nc.gpsimd.dma_start(
                        at_t[:, ci, bass.ts(j, N // 8)],
                        at_d[:, ci, bass.ts(j, N // 8)],
                    )

            def reduce_ntile(n, rmax):
                for c in range(4):
                    pst = psumt_pool.tile([128, 128], bf16, name="pst")
                    nc.tensor.transpose(
                        pst[:], rmax[:, bass.ts(c, 128)], id_t[:]
                    )
                    col = n * 4 + c
                    nc.vector.tensor_reduce(
                        res_t[:, col : col + 1], pst[:],
                        mybir.AxisListType.X, mybir.AluOpType.max,
                    )

            def compute_body():
                prev = None
                for n in range(NT):
                    rmax = rmax_pool.tile([128, 512], bf16)
                    for m in range(MT):
                        ps = psum_pool.tile([128, 512], f32)
                        for c in range(2):
                            nc.tensor.matmul(
                                ps[:],
                                bt_t[:, 2 * c : 2 * c + 2, bass.ts(m, 128)],
                                at_t[:, 2 * c : 2 * c + 2, bass.ts(n, 512)],
                                start=(c == 0),
                                stop=(c == 1),
                                perf_mode=mybir.MatmulPerfMode.DoubleRow,
                            )
                        if m == 0:
                            nc.scalar.activation(
                                rmax[:], ps[:],
                                mybir.ActivationFunctionType.Identity,
                                bias=msq_t[:, m : m + 1], scale=2.0,
                            )
                        else:
                            ev = evac_pool.tile([128, 512], bf16)
                            nc.scalar.activation(
                                ev[:], ps[:],
                                mybir.ActivationFunctionType.Identity,
                                bias=msq_t[:, m : m + 1], scale=2.0,
                            )
                            nc.vector.tensor_max(rmax[:], rmax[:], ev[:])
                        if m == 4 and prev is not None:
                            reduce_ntile(n - 1, prev)
                    prev = rmax
                reduce_ntile(NT - 1, prev)

            if repeat == 1:
                compute_body()
            else:
                with tc.For_i(0, repeat):
                    compute_body()
            nc.gpsimd.dma_start(out_d[:], res_t[:])

    nc.finalize()
    return nc


_NC = None


def prepare_in_maps(patches: np.ndarray, memory_bank: np.ndarray, fp8=True):
    m_sq = np.sum(
        memory_bank.astype(np.float64) ** 2, axis=1
    )
    C = float(np.mean(m_sq))
    id_np = np.eye(128, dtype=ml_dtypes.bfloat16)
    if fp8:
        # [128, 4, N] with dim1 = d//128 (K-chunk index)
        at_np = np.ascontiguousarray(
            patches.T.astype(ml_dtypes.float8_e4m3)
            .reshape(KP, 128, N).transpose(1, 0, 2)
        )
    else:
        at_np = np.ascontiguousarray(patches.T).astype(ml_dtypes.bfloat16)
    in_maps = []
    for c in range(N_CORES):
        bank_c = memory_bank[c * M : (c + 1) * M]
        if fp8:
            bt_np = np.ascontiguousarray(
                bank_c.T.astype(ml_dtypes.float8_e4m3)
                .reshape(KP, 128, M).transpose(1, 0, 2)
            )
        else:
            bt_np = np.ascontiguousarray(bank_c.T).astype(ml_dtypes.bfloat16)
        msq_c = -(m_sq[c * M : (c + 1) * M] - C)
        msq_np = np.ascontiguousarray(
            msq_c.reshape(MT, 128).T
        ).astype(np.float32)
        in_maps.append({"at": at_np, "bt": bt_np, "msq": msq_np, "ident": id_np})
    return in_maps


def kernel(patches: np.ndarray, memory_bank: np.ndarray) -> np.ndarray:
    global _NC
    if _NC is None:
        _NC = _build_nc()
    nc = _NC

    p64 = patches.astype(np.float64)
    b64 = memory_bank.astype(np.float64)
    p_sq = np.sum(p64 * p64, axis=1)          # [N]
    m_sq = np.sum(b64 * b64, axis=1)          # [M_TOTAL]
    C = float(np.mean(m_sq))

    in_maps = prepare_in_maps(patches, memory_bank)

    br = run_bass_kernel_spmd(nc, in_maps, list(range(N_CORES)))
    r = np.max(
        np.stack(
            [np.asarray(br.results[c]["out"], np.float64).T.reshape(N)
             for c in range(N_CORES)]
        ),
        axis=0,
    )
    min_d2 = np.maximum(p_sq + C - r, 0.0)

    # Host refinement: device min_d2 is approximate (fp8 matmul + bf16 max
    # accumulation). Recompute exact d2 rows for every candidate patch whose
    # approx score is within EPS of the max. Correctness needs
    # EPS >= 2*max|err|; measured err is +-7 (fp8e4), so 30 is ~2x margin.
    EPS = 30.0
    amax = float(min_d2.max())
    S = np.flatnonzero(min_d2 >= amax - EPS)
    if len(S) > 2048:
        S = np.argsort(min_d2)[-2048:]
    cross_S = p64[S] @ b64.T
    d2_S = p_sq[S, None] + m_sq[None, :] - 2.0 * cross_S
    score = np.sqrt(max(float(np.maximum(d2_S, 0.0).min(axis=1).max()), 0.0))
    return np.asarray(score, dtype=np.float32)
